# revision 1
# baseline (speedup 1.0000x reference)
"""KVQuantizer Trainium2 kernel.

Full input feat [1, 32, 8192, 128] fp32 is sharded head-wise across 8 cores
(4 heads/core). Per (token, head): 128-dim group quantization:
  - chunk (16 tokens) base row quantized at 8 bits (asymmetric per-group)
  - diffs vs dequantized base quantized at 4 bits + top-k pruning
    (zero the 64 smallest |deq| per group, jax top_k tie semantics)
  - out = base_deq + pruned diff_deq
"""
import os
import sys
import numpy as np

sys.path.insert(0, "/opt/trn_rl_repo")

import concourse.bass as bass
import concourse.bacc as bacc
import concourse.mybir as mybir
from concourse.tile import TileContext
from concourse.bass_utils import run_bass_kernel_spmd

F32 = mybir.dt.float32
AF = mybir.ActivationFunctionType
OP = mybir.AluOpType
AX = mybir.AxisListType

NCORES = 8
H_FULL = 32
HPC = H_FULL // NCORES   # heads per core = 4
S_FULL = 8192
D = 128
CH = 16                  # chunk size
EPS = 1e-5

MAGIC = float(np.float32(12582912.0))        # 1.5 * 2^23
H_FLOOR1 = float(np.float32(0.5 + 2**-16))   # RNE(x+this) = floor(x)+1
H_CEIL = float(np.float32(0.5 - 2**-16))     # RNE(x+this) = ceil(x)

SEL_MAX8 = os.environ.get("KVQ_SEL", "max8") == "max8"


def _quant_stats(nc, wk, x3, qmax, S_t):
    """Per-(token,head) amax/amin/scale/recip/base from x3 [128, HPC, D].
    Returns (s, rs, b) tiles [128, HPC]."""
    red = wk.tile([128, HPC, 2], F32, tag="red")
    rmax = red[:, :, 0]
    rmin = red[:, :, 1]
    nc.vector.tensor_reduce(rmax, x3, axis=AX.X, op=OP.max)
    nc.vector.tensor_reduce(rmin, x3, axis=AX.X, op=OP.min)
    sc = wk.tile([128, HPC, 2], F32, tag="scales")
    s = sc[:, :, 0]
    rs = sc[:, :, 1]
    # s = max((rmax-rmin)/qmax, EPS)
    nc.vector.tensor_tensor(s, rmax, rmin, op=OP.subtract)
    nc.vector.tensor_scalar(s, s, 1.0 / qmax, EPS, op0=OP.mult, op1=OP.max)
    nc.vector.reciprocal(rs, s)
    return s, rs, rmin


def _round_rne(nc, out, in_):
    """out = RNE-round(in_) via the fp32 magic trick (two ops, safe)."""
    nc.vector.tensor_scalar(out, in_, MAGIC, None, op0=OP.add)
    nc.vector.tensor_scalar(out, out, MAGIC, None, op0=OP.subtract)


def _quant_q(nc, wk, x3, s, rs, b, qmax, qmax_t, tag):
    """q = clip(RNE((x-b)*rs), 0, qmax), deq = q*s+b. Returns (q, deq)."""
    v = wk.tile([128, HPC, D], F32, tag=tag + "_v")
    for h in range(HPC):
        nc.vector.scalar_tensor_tensor(
            out=v[:, h], in0=x3[:, h], scalar=b[:, h : h + 1],
            in1=rs[:, h : h + 1].to_broadcast([128, D]),
            op0=OP.subtract, op1=OP.mult)
    q = wk.tile([128, HPC, D], F32, tag=tag + "_q")
    _round_rne(nc, q, v)
    # clip via two Relu passes on ScalarE: q = qmax - Relu(qmax - Relu(q))
    nc.scalar.activation(q, q, AF.Relu)
    nc.scalar.activation(q, q, AF.Relu, bias=qmax_t[: x3.shape[0]], scale=-1.0)
    nc.vector.tensor_scalar(q, q, -1.0, float(qmax), op0=OP.mult, op1=OP.add)
    deq = wk.tile([128, HPC, D], F32, tag=tag + "_deq")
    for h in range(HPC):
        nc.vector.tensor_scalar(
            deq[:, h], q[:, h], s[:, h : h + 1], b[:, h : h + 1],
            op0=OP.mult, op1=OP.add)
    return q, deq


def _select_zap_max8(nc, wk, sel, deq):
    """v1 selection: zap[p,h,d] = 1 where |deq| among 64 smallest (ties: low idx).
    Writes result into sel tiles; returns zap [128, HPC, D] (1.0 = zero it)."""
    keyn = wk.tile([128, HPC, D], F32, tag="keyn")
    nc.scalar.activation(keyn, deq, AF.Abs)
    nc.vector.tensor_scalar(keyn, keyn, -1.0, None, op0=OP.mult)  # -|deq|
    MINV = -1.0e30
    zap = wk.tile([128, HPC, D], F32, tag="zap")
    mx = wk.tile([128, 8], F32, tag="mx8")
    for h in range(HPC):
        cur = keyn[:, h]
        for it in range(64 // 8):
            nc.vector.max(out=mx, in_=cur)
            nc.vector.match_replace(
                out=zap[:, h], in_to_replace=mx, in_values=cur, imm_value=MINV)
            cur = zap[:, h]
    # zap = 1 where replaced: keyn - zap is 0 for kept, huge for replaced
    nc.vector.tensor_tensor(zap, keyn, zap, op=OP.subtract)
    nc.vector.tensor_scalar(zap, zap, 1.0, None, op0=OP.min)
    return zap


def _select_zap_bisect(nc, wk, cpool, sel_consts, deq, q, s, rs, b):
    """v2 selection via level-order index + bisection + prefix scan.
    Returns keep [128, HPC, D] (1.0 = keep)."""
    zeros128 = sel_consts
    # ch = b*rs ; biases: negch = -ch (= c/2), w-bias = 2*ch (= -c)
    t4 = wk.tile([128, HPC, 3], F32, tag="selt4")
    ch = t4[:, :, 0]
    negch = t4[:, :, 1]
    bw = t4[:, :, 2]
    nc.vector.tensor_tensor(ch, b, rs, op=OP.mult)
    nc.vector.tensor_scalar(negch, ch, -1.0, None, op0=OP.mult)
    nc.vector.tensor_scalar(bw, ch, 2.0, None, op0=OP.mult)
    # w = |2q - c| = Abs(q*2 + bw)  (per-head bias)
    w = wk.tile([128, HPC, D], F32, tag="selw")
    for h in range(HPC):
        nc.scalar.activation(w[:, h], q[:, h], AF.Abs,
                             bias=bw[:, h : h + 1], scale=2.0)
    # lo = -0.5*w + c/2 ; hi = 0.5*w + c/2   (c/2 = -ch = negch)
    lohi = wk.tile([128, 2, HPC, D], F32, tag="sellohi")
    for h in range(HPC):
        nc.scalar.activation(lohi[:, 0, h], w[:, h], AF.Identity,
                             bias=negch[:, h : h + 1], scale=-0.5)
        nc.scalar.activation(lohi[:, 1, h], w[:, h], AF.Identity,
                             bias=negch[:, h : h + 1], scale=0.5)
    # pmin = max(0, floor(lo)+1); pmax1 = min(16, ceil(hi)); phi = pmax1 - pmin
    pm = wk.tile([128, 2, HPC, D], F32, tag="selpm")
    nc.vector.tensor_scalar(pm[:, 0], lohi[:, 0], H_FLOOR1, MAGIC,
                            op0=OP.add, op1=OP.add)
    nc.vector.tensor_scalar(pm[:, 0], pm[:, 0], MAGIC, 0.0,
                            op0=OP.subtract, op1=OP.max)
    nc.vector.tensor_scalar(pm[:, 1], lohi[:, 1], H_CEIL, MAGIC,
                            op0=OP.add, op1=OP.add)
    nc.vector.tensor_scalar(pm[:, 1], pm[:, 1], MAGIC, 16.0,
                            op0=OP.subtract, op1=OP.min)
    phi = wk.tile([128, HPC, D], F32, tag="selphi")
    nc.vector.scalar_tensor_tensor(
        out=phi, in0=pm[:, 0], scalar=-1.0, in1=pm[:, 1],
        op0=OP.mult, op1=OP.add)
    # bisect t = min{p : #{phi<=p} >= 64}; phi in [-1,16]; T starts at -2
    junk = wk.tile([128, HPC, D], F32, tag="seljunk")
    tt = wk.tile([128, HPC, 4], F32, tag="selbis")
    T = tt[:, :, 0]
    Tk = tt[:, :, 1]
    cnt = tt[:, :, 2]
    dlt = tt[:, :, 3]
    nc.vector.memset(T, -2.0)
    for k in (16.0, 8.0, 4.0, 2.0, 1.0):
        nc.vector.tensor_scalar(Tk, T, k, None, op0=OP.add)
        for h in range(HPC):
            nc.vector.tensor_tensor_reduce(
                out=junk[:, h], in0=phi[:, h],
                in1=Tk[:, h : h + 1].to_broadcast([128, D]),
                scale=1.0, scalar=0.0, op0=OP.is_le, op1=OP.add,
                accum_out=cnt[:, h : h + 1])
        nc.vector.tensor_scalar(dlt, cnt, 64.0, None, op0=OP.is_lt)
        nc.vector.scalar_tensor_tensor(
            out=T, in0=dlt, scalar=k, in1=T, op0=OP.mult, op1=OP.add)
    t = Tk
    nc.vector.tensor_scalar(t, T, 1.0, None, op0=OP.add)
    # Fb = #{phi < t}; rr = 64 - Fb
    Fb = cnt
    for h in range(HPC):
        nc.vector.tensor_tensor_reduce(
            out=junk[:, h], in0=phi[:, h],
            in1=t[:, h : h + 1].to_broadcast([128, D]),
            scale=1.0, scalar=0.0, op0=OP.is_lt, op1=OP.add,
            accum_out=Fb[:, h : h + 1])
    rr = dlt
    nc.vector.tensor_scalar(rr, Fb, -1.0, 64.0, op0=OP.mult, op1=OP.add)
    # mE = [phi == t]; P = inclusive scan; zb = [P > rr]*mE; keep = [phi > t] + zb
    mE = wk.tile([128, HPC, D], F32, tag="selmE")
    P = junk
    keep = wk.tile([128, HPC, D], F32, tag="selkeep")
    for h in range(HPC):
        nc.vector.tensor_scalar(mE[:, h], phi[:, h], t[:, h : h + 1], None,
                                op0=OP.is_equal)
        nc.vector.tensor_tensor_scan(
            out=P[:, h], data0=mE[:, h], data1=zeros128,
            initial=0.0, op0=OP.add, op1=OP.add)
        nc.vector.scalar_tensor_tensor(
            out=keep[:, h], in0=P[:, h], scalar=rr[:, h : h + 1],
            in1=mE[:, h], op0=OP.is_gt, op1=OP.mult)
        nc.vector.scalar_tensor_tensor(
            out=keep[:, h], in0=phi[:, h], scalar=t[:, h : h + 1],
            in1=keep[:, h], op0=OP.is_gt, op1=OP.add)
    return keep


def build_graph(nc, S=S_FULL):
    n_chunks = S // CH
    n_btiles = (n_chunks + 127) // 128
    n_tiles = S // 128
    assert n_chunks % 128 == 0 or n_btiles == 1

    x = nc.dram_tensor("x", [HPC, S, D], F32, kind="ExternalInput")
    econst = nc.dram_tensor("econst", [128, 16 * 128], F32, kind="ExternalInput")
    rowmask_d = nc.dram_tensor("rowmask", [128, 1], F32, kind="ExternalInput")
    y = nc.dram_tensor("y", [HPC, S, D], F32, kind="ExternalOutput")

    xv = x[:, :, :].rearrange("h s d -> s h d")       # [S, HPC, D] view
    yv = y[:, :, :].rearrange("h s d -> s h d")
    xb = x[:, :, :].rearrange("h (c j) d -> c j h d", j=CH)  # [n_chunks,16,HPC,D]

    with TileContext(nc) as tc:
        with (
            tc.tile_pool(name="consts", bufs=1) as cpool,
            tc.tile_pool(name="bdq", bufs=1) as bdpool,
            tc.tile_pool(name="io", bufs=3) as io,
            tc.tile_pool(name="wk", bufs=2) as wk,
            tc.tile_pool(name="ps", bufs=2, space="PSUM") as ps,
        ):
            E_t = cpool.tile([128, 16 * 128], F32, tag="E")
            nc.sync.dma_start(E_t, econst[:, :])
            rowmask = cpool.tile([128, 1], F32, tag="rowmask")
            nc.sync.dma_start(rowmask, rowmask_d[:, :])
            zeros128 = cpool.tile([128, D], F32, tag="zeros")
            nc.vector.memset(zeros128, 0.0)
            q255_t = cpool.tile([128, 1], F32, tag="q255")
            nc.vector.memset(q255_t, 255.0)
            q15_t = cpool.tile([128, 1], F32, tag="q15")
            nc.vector.memset(q15_t, 15.0)

            # ---------- pass 1: chunk-base rows, 8-bit quant ----------
            bdeq_tiles = []
            for bt in range(n_btiles):
                nb = min(128, n_chunks - bt * 128)
                xt = io.tile([128, HPC, D], F32, tag="xin")
                nc.sync.dma_start(
                    xt[:nb], xb[bt * 128 : bt * 128 + nb, 0])
                s, rs, b = _quant_stats(nc, wk, xt[:nb], 255, None)
                q, deq = _quant_q(nc, wk, xt[:nb], s, rs, b, 255, q255_t, "b")
                bq = bdpool.tile([128, HPC, D], F32, tag=f"bdeq{bt}")
                nc.vector.tensor_copy(bq[:nb], deq)
                bdeq_tiles.append(bq)

            # ---------- pass 2: all rows as diffs, 4-bit quant + prune ----------
            for ti in range(n_tiles):
                xt = io.tile([128, HPC, D], F32, tag="xin")
                nc.sync.dma_start(xt, xv[ti * 128 : (ti + 1) * 128])
                # replicate 8 chunk-base rows -> 128 token rows via PE
                bq = bdeq_tiles[ti // 16]
                j = ti % 16
                brep = ps.tile([128, HPC * D], F32, tag="brep")
                nc.tensor.matmul(
                    brep, E_t[:, j * 128 : (j + 1) * 128],
                    bq.rearrange("p h d -> p (h d)"), start=True, stop=True)
                brep3 = brep.rearrange("p (h d) -> p h d", d=D)
                diff = wk.tile([128, HPC, D], F32, tag="diff")
                nc.vector.tensor_tensor(diff, xt, brep3, op=OP.subtract)
                s, rs, b = _quant_stats(nc, wk, diff, 15, None)
                q, deq = _quant_q(nc, wk, diff, s, rs, b, 15, q15_t, "d")
                if SEL_MAX8:
                    zap = _select_zap_max8(nc, wk, None, deq)
                    dq = wk.tile([128, HPC, D], F32, tag="dq")
                    nc.vector.scalar_tensor_tensor(
                        out=dq, in0=zap, scalar=0.0, in1=deq,
                        op0=OP.is_equal, op1=OP.mult)
                else:
                    keep = _select_zap_bisect(nc, wk, cpool, zeros128,
                                              deq, q, s, rs, b)
                    dq = wk.tile([128, HPC, D], F32, tag="dq")
                    nc.vector.tensor_tensor(dq, keep, deq, op=OP.mult)
                # rowmask zeroes diffq on chunk-base token rows; add base
                outt = io.tile([128, HPC, D], F32, tag="outt")
                for h in range(HPC):
                    nc.vector.scalar_tensor_tensor(
                        out=outt[:, h], in0=dq[:, h], scalar=rowmask,
                        in1=brep3[:, h], op0=OP.mult, op1=OP.add)
                nc.sync.dma_start(yv[ti * 128 : (ti + 1) * 128], outt)
    return nc


def _make_consts():
    # E[k, j*128 + m] = 1 where k == 8*j + m//16: expander for chunk-group j
    E = np.zeros((128, 16 * 128), np.float32)
    for j in range(16):
        for m in range(128):
            E[8 * j + m // 16, j * 128 + m] = 1.0
    rowmask = np.ones((128, 1), np.float32)
    rowmask[0::16] = 0.0
    return E, rowmask


_GRAPH_CACHE = {}


def _get_graph(S):
    if S not in _GRAPH_CACHE:
        nc = bacc.Bacc()
        build_graph(nc, S)
        nc.finalize()
        _GRAPH_CACHE[S] = nc
    return _GRAPH_CACHE[S]


def kernel(feat, diff_len):
    feat = np.asarray(feat)
    diff_len = int(diff_len)
    assert feat.shape == (1, H_FULL, S_FULL, D)
    E, rowmask = _make_consts()
    nc = _get_graph(S_FULL)
    in_maps = []
    for c in range(NCORES):
        shard = np.ascontiguousarray(feat[0, c * HPC : (c + 1) * HPC], np.float32)
        in_maps.append({"x": shard, "econst": E, "rowmask": rowmask})
    res = run_bass_kernel_spmd(nc, in_maps, core_ids=list(range(NCORES)))
    outs = [np.asarray(r["y"]).reshape(HPC, S_FULL, D) for r in res.results]
    full = np.concatenate(outs, axis=0)[None]   # [1, 32, 8192, 128]
    if diff_len < S_FULL:
        full = full.copy()
        full[:, :, diff_len:] = feat[:, :, diff_len:]
    return full.astype(feat.dtype, copy=False)



# revision 2
# speedup vs baseline: 10305.9298x; 10305.9298x over previous
"""KVQuantizer Trainium2 kernel.

Full input feat [1, 32, 8192, 128] fp32 is sharded head-wise across 8 cores
(4 heads/core). Per (token, head): 128-dim group quantization:
  - chunk (16 tokens) base row quantized at 8 bits (asymmetric per-group)
  - diffs vs dequantized base quantized at 4 bits + top-k pruning
    (zero the 64 smallest |deq| per group, jax top_k tie semantics)
  - out = base_deq + pruned diff_deq
"""
import os
import sys
import numpy as np

sys.path.insert(0, "/opt/trn_rl_repo")

import concourse.bass as bass
import concourse.bacc as bacc
import concourse.mybir as mybir
from concourse.tile import TileContext
from concourse.bass_utils import run_bass_kernel_spmd

F32 = mybir.dt.float32
AF = mybir.ActivationFunctionType
OP = mybir.AluOpType
AX = mybir.AxisListType

NCORES = 8
H_FULL = 32
HPC = H_FULL // NCORES   # heads per core = 4
S_FULL = 8192
D = 128
CH = 16                  # chunk size
EPS = 1e-5

MAGIC = float(np.float32(12582912.0))        # 1.5 * 2^23
H_FLOOR1 = float(np.float32(0.5 + 2**-16))   # RNE(x+this) = floor(x)+1
H_CEIL = float(np.float32(0.5 - 2**-16))     # RNE(x+this) = ceil(x)

SEL_MAX8 = os.environ.get("KVQ_SEL", "max8") == "max8"


def _quant_stats(nc, wk, x3, qmax, S_t):
    """Per-(token,head) amax/amin/scale/recip/base from x3 [128, HPC, D].
    Returns (s, rs, b) tiles [128, HPC]."""
    red = wk.tile([128, HPC, 2], F32, tag="red")
    rmax = red[:, :, 0]
    rmin = red[:, :, 1]
    nc.vector.tensor_reduce(rmax, x3, axis=AX.X, op=OP.max)
    nc.vector.tensor_reduce(rmin, x3, axis=AX.X, op=OP.min)
    sc = wk.tile([128, HPC, 2], F32, tag="scales")
    s = sc[:, :, 0]
    rs = sc[:, :, 1]
    # s = max((rmax-rmin)/qmax, EPS)
    nc.vector.tensor_tensor(s, rmax, rmin, op=OP.subtract)
    nc.vector.tensor_scalar(s, s, 1.0 / qmax, EPS, op0=OP.mult, op1=OP.max)
    nc.vector.reciprocal(rs, s)
    return s, rs, rmin


def _round_rne(nc, out, in_):
    """out = RNE-round(in_) via the fp32 magic trick (two ops, safe)."""
    nc.vector.tensor_scalar(out, in_, MAGIC, None, op0=OP.add)
    nc.vector.tensor_scalar(out, out, MAGIC, None, op0=OP.subtract)


def _quant_q(nc, wk, x3, s, rs, b, qmax, qmax_t, tag):
    """q = clip(RNE((x-b)*rs), 0, qmax), deq = q*s+b. Returns (q, deq)."""
    v = wk.tile([128, HPC, D], F32, tag=tag + "_v")
    for h in range(HPC):
        nc.vector.scalar_tensor_tensor(
            out=v[:, h], in0=x3[:, h], scalar=b[:, h : h + 1],
            in1=rs[:, h : h + 1].to_broadcast([128, D]),
            op0=OP.subtract, op1=OP.mult)
    q = wk.tile([128, HPC, D], F32, tag=tag + "_q")
    _round_rne(nc, q, v)
    # clip via two Relu passes on ScalarE: q = qmax - Relu(qmax - Relu(q))
    nc.scalar.activation(q, q, AF.Relu)
    nc.scalar.activation(q, q, AF.Relu, bias=qmax_t[: x3.shape[0]], scale=-1.0)
    nc.vector.tensor_scalar(q, q, -1.0, float(qmax), op0=OP.mult, op1=OP.add)
    deq = wk.tile([128, HPC, D], F32, tag=tag + "_deq")
    for h in range(HPC):
        nc.vector.tensor_scalar(
            deq[:, h], q[:, h], s[:, h : h + 1], b[:, h : h + 1],
            op0=OP.mult, op1=OP.add)
    return q, deq


def _select_zap_max8(nc, wk, sel, deq):
    """v1 selection: zap[p,h,d] = 1 where |deq| among 64 smallest (ties: low idx).
    Writes result into sel tiles; returns zap [128, HPC, D] (1.0 = zero it)."""
    keyn = wk.tile([128, HPC, D], F32, tag="keyn")
    nc.scalar.activation(keyn, deq, AF.Abs)
    nc.vector.tensor_scalar(keyn, keyn, -1.0, None, op0=OP.mult)  # -|deq|
    MINV = -1.0e30
    zap = wk.tile([128, HPC, D], F32, tag="zap")
    mx = wk.tile([128, 8], F32, tag="mx8")
    for h in range(HPC):
        cur = keyn[:, h]
        for it in range(64 // 8):
            nc.vector.max(out=mx, in_=cur)
            nc.vector.match_replace(
                out=zap[:, h], in_to_replace=mx, in_values=cur, imm_value=MINV)
            cur = zap[:, h]
    # zap = 1 where replaced: keyn - zap is 0 for kept, huge for replaced
    nc.vector.tensor_tensor(zap, keyn, zap, op=OP.subtract)
    nc.vector.tensor_scalar(zap, zap, 1.0, None, op0=OP.min)
    return zap


def _select_zap_bisect(nc, wk, cpool, sel_consts, deq, q, s, rs, b):
    """v2 selection via level-order index + bisection + prefix scan.
    Returns keep [128, HPC, D] (1.0 = keep)."""
    zeros128 = sel_consts
    # ch = b*rs ; biases: negch = -ch (= c/2), w-bias = 2*ch (= -c)
    t4 = wk.tile([128, HPC, 3], F32, tag="selt4")
    ch = t4[:, :, 0]
    negch = t4[:, :, 1]
    bw = t4[:, :, 2]
    nc.vector.tensor_tensor(ch, b, rs, op=OP.mult)
    nc.vector.tensor_scalar(negch, ch, -1.0, None, op0=OP.mult)
    nc.vector.tensor_scalar(bw, ch, 2.0, None, op0=OP.mult)
    # w = |2q - c| = Abs(q*2 + bw)  (per-head bias)
    w = wk.tile([128, HPC, D], F32, tag="selw")
    for h in range(HPC):
        nc.scalar.activation(w[:, h], q[:, h], AF.Abs,
                             bias=bw[:, h : h + 1], scale=2.0)
    # lo = -0.5*w + c/2 ; hi = 0.5*w + c/2   (c/2 = -ch = negch)
    lohi = wk.tile([128, 2, HPC, D], F32, tag="sellohi")
    for h in range(HPC):
        nc.scalar.activation(lohi[:, 0, h], w[:, h], AF.Identity,
                             bias=negch[:, h : h + 1], scale=-0.5)
        nc.scalar.activation(lohi[:, 1, h], w[:, h], AF.Identity,
                             bias=negch[:, h : h + 1], scale=0.5)
    # pmin = max(0, floor(lo)+1); pmax1 = min(16, ceil(hi)); phi = pmax1 - pmin
    pm = wk.tile([128, 2, HPC, D], F32, tag="selpm")
    nc.vector.tensor_scalar(pm[:, 0], lohi[:, 0], H_FLOOR1, MAGIC,
                            op0=OP.add, op1=OP.add)
    nc.vector.tensor_scalar(pm[:, 0], pm[:, 0], MAGIC, 0.0,
                            op0=OP.subtract, op1=OP.max)
    nc.vector.tensor_scalar(pm[:, 1], lohi[:, 1], H_CEIL, MAGIC,
                            op0=OP.add, op1=OP.add)
    nc.vector.tensor_scalar(pm[:, 1], pm[:, 1], MAGIC, 16.0,
                            op0=OP.subtract, op1=OP.min)
    phi = wk.tile([128, HPC, D], F32, tag="selphi")
    nc.vector.scalar_tensor_tensor(
        out=phi, in0=pm[:, 0], scalar=-1.0, in1=pm[:, 1],
        op0=OP.mult, op1=OP.add)
    # bisect t = min{p : #{phi<=p} >= 64}; phi in [-1,16]; T starts at -2
    junk = wk.tile([128, HPC, D], F32, tag="seljunk")
    tt = wk.tile([128, HPC, 4], F32, tag="selbis")
    T = tt[:, :, 0]
    Tk = tt[:, :, 1]
    cnt = tt[:, :, 2]
    dlt = tt[:, :, 3]
    nc.vector.memset(T, -2.0)
    for k in (16.0, 8.0, 4.0, 2.0, 1.0):
        nc.vector.tensor_scalar(Tk, T, k, None, op0=OP.add)
        for h in range(HPC):
            nc.vector.tensor_tensor_reduce(
                out=junk[:, h], in0=phi[:, h],
                in1=Tk[:, h : h + 1].to_broadcast([128, D]),
                scale=1.0, scalar=0.0, op0=OP.is_le, op1=OP.add,
                accum_out=cnt[:, h : h + 1])
        nc.vector.tensor_scalar(dlt, cnt, 64.0, None, op0=OP.is_lt)
        nc.vector.scalar_tensor_tensor(
            out=T, in0=dlt, scalar=k, in1=T, op0=OP.mult, op1=OP.add)
    t = Tk
    nc.vector.tensor_scalar(t, T, 1.0, None, op0=OP.add)
    # Fb = #{phi < t}; rr = 64 - Fb
    Fb = cnt
    for h in range(HPC):
        nc.vector.tensor_tensor_reduce(
            out=junk[:, h], in0=phi[:, h],
            in1=t[:, h : h + 1].to_broadcast([128, D]),
            scale=1.0, scalar=0.0, op0=OP.is_lt, op1=OP.add,
            accum_out=Fb[:, h : h + 1])
    rr = dlt
    nc.vector.tensor_scalar(rr, Fb, -1.0, 64.0, op0=OP.mult, op1=OP.add)
    # mE = [phi == t]; P = inclusive scan; zb = [P > rr]*mE; keep = [phi > t] + zb
    mE = wk.tile([128, HPC, D], F32, tag="selmE")
    P = junk
    keep = wk.tile([128, HPC, D], F32, tag="selkeep")
    for h in range(HPC):
        nc.vector.tensor_scalar(mE[:, h], phi[:, h], t[:, h : h + 1], None,
                                op0=OP.is_equal)
        nc.vector.tensor_tensor_scan(
            out=P[:, h], data0=mE[:, h], data1=zeros128,
            initial=0.0, op0=OP.add, op1=OP.add)
        nc.vector.scalar_tensor_tensor(
            out=keep[:, h], in0=P[:, h], scalar=rr[:, h : h + 1],
            in1=mE[:, h], op0=OP.is_gt, op1=OP.mult)
        nc.vector.scalar_tensor_tensor(
            out=keep[:, h], in0=phi[:, h], scalar=t[:, h : h + 1],
            in1=keep[:, h], op0=OP.is_gt, op1=OP.add)
    return keep


def build_graph(nc, S=S_FULL):
    n_chunks = S // CH
    n_btiles = (n_chunks + 127) // 128
    n_tiles = S // 128
    assert n_chunks % 128 == 0 or n_btiles == 1

    x = nc.dram_tensor("x", [HPC, S, D], F32, kind="ExternalInput")
    econst = nc.dram_tensor("econst", [128, 16 * 128], F32, kind="ExternalInput")
    rowmask_d = nc.dram_tensor("rowmask", [128, 1], F32, kind="ExternalInput")
    y = nc.dram_tensor("y", [HPC, S, D], F32, kind="ExternalOutput")

    xv = x[:, :, :].rearrange("h s d -> s h d")       # [S, HPC, D] view
    yv = y[:, :, :].rearrange("h s d -> s h d")
    xb = x[:, :, :].rearrange("h (c j) d -> c j h d", j=CH)  # [n_chunks,16,HPC,D]

    with TileContext(nc) as tc:
        with (
            tc.tile_pool(name="consts", bufs=1) as cpool,
            tc.tile_pool(name="bdq", bufs=1) as bdpool,
            tc.tile_pool(name="io", bufs=3) as io,
            tc.tile_pool(name="wk", bufs=2) as wk,
            tc.tile_pool(name="ps", bufs=2, space="PSUM") as ps,
        ):
            E_t = cpool.tile([128, 16 * 128], F32, tag="E")
            nc.sync.dma_start(E_t, econst[:, :])
            rowmask = cpool.tile([128, 1], F32, tag="rowmask")
            nc.sync.dma_start(rowmask, rowmask_d[:, :])
            zeros128 = cpool.tile([128, D], F32, tag="zeros")
            nc.vector.memset(zeros128, 0.0)
            q255_t = cpool.tile([128, 1], F32, tag="q255")
            nc.vector.memset(q255_t, 255.0)
            q15_t = cpool.tile([128, 1], F32, tag="q15")
            nc.vector.memset(q15_t, 15.0)

            # ---------- pass 1: chunk-base rows, 8-bit quant ----------
            bdeq_tiles = []
            for bt in range(n_btiles):
                nb = min(128, n_chunks - bt * 128)
                xt = io.tile([128, HPC, D], F32, tag="xin")
                nc.sync.dma_start(
                    xt[:nb], xb[bt * 128 : bt * 128 + nb, 0])
                s, rs, b = _quant_stats(nc, wk, xt[:nb], 255, None)
                q, deq = _quant_q(nc, wk, xt[:nb], s, rs, b, 255, q255_t, "b")
                bq = bdpool.tile([128, HPC, D], F32, tag=f"bdeq{bt}")
                nc.vector.tensor_copy(bq[:nb], deq)
                bdeq_tiles.append(bq)

            # ---------- pass 2: all rows as diffs, 4-bit quant + prune ----------
            for ti in range(n_tiles):
                xt = io.tile([128, HPC, D], F32, tag="xin")
                nc.sync.dma_start(xt, xv[ti * 128 : (ti + 1) * 128])
                # replicate 8 chunk-base rows -> 128 token rows via PE
                bq = bdeq_tiles[ti // 16]
                j = ti % 16
                brep = ps.tile([128, HPC * D], F32, tag="brep")
                nc.tensor.matmul(
                    brep, E_t[:, j * 128 : (j + 1) * 128],
                    bq.rearrange("p h d -> p (h d)"), start=True, stop=True)
                brep3 = brep.rearrange("p (h d) -> p h d", d=D)
                diff = wk.tile([128, HPC, D], F32, tag="diff")
                nc.vector.tensor_tensor(diff, xt, brep3, op=OP.subtract)
                s, rs, b = _quant_stats(nc, wk, diff, 15, None)
                q, deq = _quant_q(nc, wk, diff, s, rs, b, 15, q15_t, "d")
                if SEL_MAX8:
                    zap = _select_zap_max8(nc, wk, None, deq)
                    dq = wk.tile([128, HPC, D], F32, tag="dq")
                    nc.vector.scalar_tensor_tensor(
                        out=dq, in0=zap, scalar=0.0, in1=deq,
                        op0=OP.is_equal, op1=OP.mult)
                else:
                    keep = _select_zap_bisect(nc, wk, cpool, zeros128,
                                              deq, q, s, rs, b)
                    dq = wk.tile([128, HPC, D], F32, tag="dq")
                    nc.vector.tensor_tensor(dq, keep, deq, op=OP.mult)
                # rowmask zeroes diffq on chunk-base token rows; add base
                outt = io.tile([128, HPC, D], F32, tag="outt")
                for h in range(HPC):
                    nc.vector.scalar_tensor_tensor(
                        out=outt[:, h], in0=dq[:, h], scalar=rowmask,
                        in1=brep3[:, h], op0=OP.mult, op1=OP.add)
                nc.sync.dma_start(yv[ti * 128 : (ti + 1) * 128], outt)
    return nc


def _make_consts():
    # E[k, j*128 + m] = 1 where k == 8*j + m//16: expander for chunk-group j
    E = np.zeros((128, 16 * 128), np.float32)
    for j in range(16):
        for m in range(128):
            E[8 * j + m // 16, j * 128 + m] = 1.0
    rowmask = np.ones((128, 1), np.float32)
    rowmask[0::16] = 0.0
    return E, rowmask


_GRAPH_CACHE = {}


def _get_graph(S):
    if S not in _GRAPH_CACHE:
        nc = bacc.Bacc()
        build_graph(nc, S)
        nc.finalize()
        _GRAPH_CACHE[S] = nc
    return _GRAPH_CACHE[S]


LAST_RESULT = None


def kernel(feat, diff_len):
    global LAST_RESULT
    feat = np.asarray(feat)
    diff_len = int(diff_len)
    assert feat.shape == (1, H_FULL, S_FULL, D)
    E, rowmask = _make_consts()
    nc = _get_graph(S_FULL)
    in_maps = []
    for c in range(NCORES):
        shard = np.ascontiguousarray(feat[0, c * HPC : (c + 1) * HPC], np.float32)
        in_maps.append({"x": shard, "econst": E, "rowmask": rowmask})
    tkw = {}
    if os.environ.get("KVQ_TRACE"):
        tkw = dict(trace=True, tmpdir=os.environ.get("KVQ_TRACE_DIR") or None)
    res = run_bass_kernel_spmd(nc, in_maps, core_ids=list(range(NCORES)), **tkw)
    LAST_RESULT = res
    outs = [np.asarray(r["y"]).reshape(HPC, S_FULL, D) for r in res.results]
    full = np.concatenate(outs, axis=0)[None]   # [1, 32, 8192, 128]
    if diff_len < S_FULL:
        full = full.copy()
        full[:, :, diff_len:] = feat[:, :, diff_len:]
    return full.astype(feat.dtype, copy=False)



# revision 7
# speedup vs baseline: 19859.3834x; 1.9270x over previous
"""KVQuantizer Trainium2 kernel (v2 — custom-DVE selection).

Full input feat [1, 32, 8192, 128] fp32 sharded head-wise across 8 cores
(4 heads/core). Per (token, head): 128-dim group quantization:
  - chunk (16 tokens) base row quantized at 8 bits (asymmetric per-group)
  - diffs vs dequantized base quantized at 4 bits + top-k pruning
    (zero the 64 smallest-|deq| per group, jax top_k tie semantics)
  - out = base_deq + pruned diff_deq

Selection strategy: |deq| = s*|q - c| with c = -b/s, so ranking reduces to
an integer "level index" kappa = 2*m + side-bit per element (m = integer
distance from c), computed per head by a fused custom-DVE op. The 64-of-128
threshold T* is found from packed 3x8-bit CDF counts (two rounds), and the
exact tie order (lowest index first) comes from an in-op prefix scan.
Verified bit-exact against jax top_k on the seed-0 dataset.
"""
import os
import sys
import numpy as np

sys.path.insert(0, "/opt/trn_rl_repo")

import concourse.bass as bass
import concourse.bacc as bacc
import concourse.mybir as mybir
from concourse.tile import TileContext
from concourse.bass_utils import run_bass_kernel_spmd
from concourse import dve_ops
from concourse.dve_ops import DveOp
from concourse.dve_uop import DveOpSpec
from concourse.dve_spec import (
    Spec, Src0, Src1, C0, C1, C2, C3, Zero, One, relu, sq, minn, select,
    lower, AluOp, scan, _spill_c3_to_src1, _has_src1,
)

F32 = mybir.dt.float32
F16 = mybir.dt.float16
I32 = mybir.dt.int32
AF = mybir.ActivationFunctionType
OP = mybir.AluOpType
AX = mybir.AxisListType

NCORES = 8
H_FULL = 32
HPC = H_FULL // NCORES   # heads per core = 4
S_FULL = 8192
D = 128
CH = 16                  # chunk size
EPS = 1e-5
NB = 16                  # tiles per super-batch
MAGIC = float(np.float32(12582912.0))        # 1.5 * 2^23
H_CEIL = float(np.float32(0.5 - 2**-16))     # RNE(x+this) = ceil(x)


# ---------------------------------------------------------------------------
# custom DVE ops
# ---------------------------------------------------------------------------

def _mk_op(name, spec, subdim=False):
    if name in dve_ops._SUB_OPCODE_FOR_NAME:
        for op in dve_ops.OPS:
            if op.name == name:
                return op
    shas = {}
    for ver in ("v3", "v4"):
        s = DveOpSpec(name=name, opcode=1, uops=lower(spec, ver=ver),
                      rd1_en=_has_src1(spec))
        shas[ver] = s.sha(ver)
    op = DveOp(name, spec, subdim, shas)
    dve_ops.OPS.append(op)
    dve_ops._SUB_OPCODE_FOR_NAME[name] = (
        dve_ops._CUSTOM_DVE_ROW_BASE + len(dve_ops.OPS) - 1)
    dve_ops.CUSTOM_DVE_SPECS[name] = op.spec
    assert dve_ops._SUB_OPCODE_FOR_NAME[name] < 0x20
    return op


def _ref_q(in0, in1, s0, s1, imm2):
    v = (in0.astype(np.float32) - s0) * s1
    r = ((v + imm2) - imm2).astype(np.float32)
    return np.minimum(np.maximum(r, 0.0), in1).astype(np.float32)


def _ref_kappa(in0, in1, s0, s1, imm2):
    t = in0.astype(np.float32) * 2.0
    u1 = t - s0
    u2 = s1 - t
    return np.where(u1 >= 0, u1, u2).astype(np.float32)


def _ref_cnt3i(in0, in1, s0, s1, imm2):
    x = in0.astype(np.float32)
    e = ((x <= s0) + ((x <= s1) + (x <= imm2) * in1) * in1).astype(np.float32)
    return e, e.reshape(e.shape[0], -1).sum(-1, keepdims=True)


def _ref_cnt3r(in0, in1, s0, s1, imm2):
    x = in0.astype(np.float32)
    e = ((x <= s0) + ((x <= s0 - 1) + (x <= s1) * imm2) * imm2).astype(np.float32)
    return e, e.reshape(e.shape[0], -1).sum(-1, keepdims=True)


def _ref_keep(in0, in1, s0, s1, imm2):
    x = in0.astype(np.float32)
    a_le = (x <= s0).astype(np.float32)
    b_lt = (x <= s1).astype(np.float32)
    tie = a_le - b_lt
    P = np.cumsum(tie, axis=-1)
    return (1.0 - (b_lt + tie * (P <= in1))).astype(np.float32)


# q = clip(rne((diff - b) * rs), 0, 15); C0=b, C1=rs, C2=MAGIC, Src1=15-const
KVQ_Q = _mk_op("KVQ_Q", Spec(
    body=_spill_c3_to_src1(minn(relu(((Src0 - C0) * C1 + C2) - C2), C3)),
    reference=_ref_q))

# kappa = select(2q - A >= 0, 2q - A, B - 2q); C0=A, C1=B
_t2 = Src0 + Src0
_u1 = _t2 - C0
_u2 = C1 - _t2
KVQ_KAPPA = _mk_op("KVQ_KAPPA", Spec(
    body=select(_u1 >= Zero, _u1, _u2),
    reference=_ref_kappa))

# packed counts at imm thresholds C0, C1, C2 (fields 1, 256, 65536); Src1=256
KVQ_CNT3I = _mk_op("KVQ_CNT3I", Spec(
    body=_spill_c3_to_src1(
        (Src0 <= C0) + ((Src0 <= C1) + (Src0 <= C2) * C3) * C3),
    accum=AluOp.ADD,
    reference=_ref_cnt3i))

# packed counts at AP thresholds C0, C0-1, C1; multiplier C2 (=256 imm)
KVQ_CNT3R = _mk_op("KVQ_CNT3R", Spec(
    body=(Src0 <= C0) + ((Src0 <= C0 - One) + (Src0 <= C1) * C2) * C2,
    accum=AluOp.ADD,
    reference=_ref_cnt3r))

# keep = 1 - (b_lt + tie * (prefix_count(tie) <= R)); C0=T*, C1=T*-1, Src1=R
_a_le = Src0 <= C0
_b_lt = Src0 <= C1
_tie = _a_le - _b_lt
_P = scan(AluOp.ADD, _tie)
KVQ_KEEP = _mk_op("KVQ_KEEP", Spec(
    body=_spill_c3_to_src1(One - (_b_lt + _tie * (_P <= C3))),
    reference=_ref_keep))


# ---------------------------------------------------------------------------
# graph
# ---------------------------------------------------------------------------

def build_graph(nc, S=S_FULL):
    n_chunks = S // CH
    n_btiles = (n_chunks + 127) // 128
    n_tiles = S // 128
    n_sb = (n_tiles + NB - 1) // NB
    assert n_tiles % NB == 0

    x = nc.dram_tensor("x", [HPC, S, D], F32, kind="ExternalInput")
    econst = nc.dram_tensor("econst", [128, 16 * 128], F32, kind="ExternalInput")
    rowmask_d = nc.dram_tensor("rowmask", [128, 1], F32, kind="ExternalInput")
    y = nc.dram_tensor("y", [HPC, S, D], F32, kind="ExternalOutput")

    xv = x[:, :, :].rearrange("h s d -> s h d")
    yv = y[:, :, :].rearrange("h s d -> s h d")
    xb = x[:, :, :].rearrange("h (c j) d -> c j h d", j=CH)

    with TileContext(nc) as tc:
        with (
            tc.tile_pool(name="consts", bufs=1) as cpool,
            tc.tile_pool(name="bdq", bufs=1) as bdpool,
            tc.tile_pool(name="io", bufs=3) as io,
            tc.tile_pool(name="wk", bufs=2) as wk,
            tc.tile_pool(name="diffp", bufs=1) as diffp,
            tc.tile_pool(name="qp", bufs=2) as qp,
            tc.tile_pool(name="kp", bufs=2) as kp,
            tc.tile_pool(name="sm", bufs=2) as sm,
            tc.tile_pool(name="tr", bufs=3) as tr,
            tc.tile_pool(name="ps", bufs=3, space="PSUM") as ps,
        ):
            E_t = cpool.tile([128, 16 * 128], F32, tag="E")
            nc.sync.dma_start(E_t, econst[:, :])
            rowmask = cpool.tile([128, 1], F32, tag="rowmask")
            nc.sync.dma_start(rowmask, rowmask_d[:, :])
            c15 = cpool.tile([128, 1], F32, tag="c15")
            nc.vector.memset(c15, 15.0)
            c256 = cpool.tile([128, 1], F32, tag="c256")
            nc.vector.memset(c256, 256.0)
            q255_t = cpool.tile([128, 1], F32, tag="q255")
            nc.vector.memset(q255_t, 255.0)

            # ---------- pass 0: chunk-base rows, 8-bit quant (as v1) ----------
            bdeq_tiles = []
            for bt in range(n_btiles):
                nb = min(128, n_chunks - bt * 128)
                xt = io.tile([128, HPC, D], F32, tag="xin")
                nc.sync.dma_start(xt[:nb], xb[bt * 128: bt * 128 + nb, 0])
                red = wk.tile([128, HPC, 2], F32, tag="bred")
                rmax = red[:, :, 0]
                rmin = red[:, :, 1]
                nc.vector.tensor_reduce(rmax, xt[:nb], axis=AX.X, op=OP.max)
                nc.vector.tensor_reduce(rmin, xt[:nb], axis=AX.X, op=OP.min)
                sc = wk.tile([128, HPC, 2], F32, tag="bscales")
                s_b = sc[:, :, 0]
                rs_b = sc[:, :, 1]
                nc.vector.tensor_tensor(s_b, rmax, rmin, op=OP.subtract)
                nc.vector.tensor_scalar(s_b, s_b, 1.0 / 255.0, EPS,
                                        op0=OP.mult, op1=OP.max)
                nc.vector.reciprocal(rs_b, s_b)
                qt = wk.tile([128, HPC, D], F32, tag="bq")
                for h in range(HPC):
                    nc.vector._custom_dve(
                        KVQ_Q, out=qt[:nb, h], in0=xt[:nb, h],
                        in1=q255_t[:nb], s0=rmin[:, h: h + 1],
                        s1=rs_b[:, h: h + 1], imm2=MAGIC)
                bq = bdpool.tile([128, HPC, D], F32, tag=f"bdeq{bt}")
                for h in range(HPC):
                    nc.vector.tensor_scalar(
                        bq[:nb, h], qt[:nb, h], s_b[:, h: h + 1],
                        rmin[:, h: h + 1], op0=OP.mult, op1=OP.add)
                bdeq_tiles.append(bq)

            # ---------- main: super-batches of NB tiles ----------
            for sb in range(n_sb):
                t0 = sb * NB
                # stats tiles for this batch
                stq = sm.tile([128, NB, HPC, 2], F32, tag="stq")   # mx, mn
                diffs = []
                for i in range(NB):
                    ti = t0 + i
                    xt = io.tile([128, HPC, D], F32, tag="xin")
                    nc.sync.dma_start(xt, xv[ti * 128: (ti + 1) * 128])
                    bqt = bdeq_tiles[ti // 16]
                    j = ti % 16
                    brep = ps.tile([128, HPC * D], F32, tag="brep")
                    nc.tensor.matmul(
                        brep, E_t[:, j * 128: (j + 1) * 128],
                        bqt.rearrange("p h d -> p (h d)"), start=True, stop=True)
                    dt = diffp.tile([128, HPC, D], F32, tag=f"diff{i}")
                    nc.vector.tensor_tensor(
                        dt, xt, brep.rearrange("p (h d) -> p h d", d=D),
                        op=OP.subtract)
                    nc.vector.tensor_reduce(stq[:, i, :, 0], dt, axis=AX.X,
                                            op=OP.max)
                    nc.vector.tensor_reduce(stq[:, i, :, 1], dt, axis=AX.X,
                                            op=OP.min)
                    diffs.append(dt)

                # ---- batched smalls A: quant chain ----
                mx = stq[:, :, :, 0]
                mn = stq[:, :, :, 1]
                sA = sm.tile([128, NB, HPC, 4], F32, tag="sA")  # s, rs, c, _
                s_ = sA[:, :, :, 0]
                rs = sA[:, :, :, 1]
                c = sA[:, :, :, 2]
                nc.vector.tensor_tensor(s_, mx, mn, op=OP.subtract)
                nc.vector.tensor_scalar(s_, s_, 1.0 / 15.0, EPS,
                                        op0=OP.mult, op1=OP.max)
                nc.vector.reciprocal(rs, s_)
                nc.vector.scalar_tensor_tensor(
                    out=c, in0=mn, scalar=-1.0, in1=rs, op0=OP.mult, op1=OP.mult)
                sB = sm.tile([128, NB, HPC, 8], F32, tag="sB")
                ff = sB[:, :, :, 0]
                cc = sB[:, :, :, 1]
                f1 = sB[:, :, :, 2]
                f2 = sB[:, :, :, 3]
                gt = sB[:, :, :, 4]
                intc = sB[:, :, :, 5]
                AA = sB[:, :, :, 6]
                BB = sB[:, :, :, 7]
                nc.vector.tensor_scalar(ff, c, -H_CEIL, MAGIC, op0=OP.add,
                                        op1=OP.add)
                nc.vector.tensor_scalar(ff, ff, MAGIC, None, op0=OP.subtract)
                nc.vector.tensor_scalar(cc, c, H_CEIL, MAGIC, op0=OP.add,
                                        op1=OP.add)
                nc.vector.tensor_scalar(cc, cc, MAGIC, None, op0=OP.subtract)
                nc.vector.tensor_tensor(f1, cc, c, op=OP.subtract)
                nc.vector.tensor_tensor(f2, c, ff, op=OP.subtract)
                nc.vector.tensor_tensor(gt, f1, f2, op=OP.is_gt)
                # eqf reuses f2 slot after use; intc = eqf * (f1 == 0)
                eqf = sA[:, :, :, 3]
                nc.vector.tensor_tensor(eqf, f1, f2, op=OP.is_equal)
                nc.vector.tensor_scalar(intc, f1, 0.0, None, op0=OP.is_equal)
                nc.vector.tensor_tensor(intc, eqf, intc, op=OP.mult)
                # halfc = eqf - intc ; bA = gt + 0.5*halfc ; bB = bA + intc
                halfc = f2
                nc.vector.tensor_tensor(halfc, eqf, intc, op=OP.subtract)
                bA = f1
                nc.vector.scalar_tensor_tensor(
                    out=bA, in0=halfc, scalar=0.5, in1=gt, op0=OP.mult, op1=OP.add)
                bBm1 = gt
                nc.vector.tensor_tensor(bBm1, bA, intc, op=OP.add)
                nc.vector.tensor_scalar(bBm1, bBm1, 1.0, None, op0=OP.subtract)
                nc.vector.scalar_tensor_tensor(
                    out=AA, in0=cc, scalar=2.0, in1=bA, op0=OP.mult,
                    op1=OP.subtract)
                nc.vector.scalar_tensor_tensor(
                    out=BB, in0=ff, scalar=2.0, in1=bBm1, op0=OP.mult,
                    op1=OP.subtract)

                # ---- P2a: q, kappa, count round 1 ----
                Z1 = sm.tile([128, NB, HPC], F32, tag="Z1")
                qs = []
                ks = []
                junk = tr.tile([128, D], F16, tag="cjunk")
                for i in range(NB):
                    dt = diffs[i]
                    qt = qp.tile([128, HPC, D], F16, tag=f"q{i}")
                    kt = kp.tile([128, HPC, D], F16, tag=f"k{i}")
                    for h in range(HPC):
                        nc.vector._custom_dve(
                            KVQ_Q, out=qt[:, h], in0=dt[:, h], in1=c15,
                            s0=mn[:, i, h: h + 1], s1=rs[:, i, h: h + 1],
                            imm2=MAGIC)
                    for h in range(HPC):
                        nc.vector._custom_dve(
                            KVQ_KAPPA, out=kt[:, h], in0=qt[:, h],
                            s0=AA[:, i, h: h + 1], s1=BB[:, i, h: h + 1])
                    for h in range(HPC):
                        nc.vector._custom_dve(
                            KVQ_CNT3I, out=junk, in0=kt[:, h], in1=c256,
                            s0=1.0, s1=3.0, imm2=5.0,
                            accum_out=Z1[:, i, h: h + 1])
                    qs.append(qt)
                    ks.append(kt)

                # ---- batched smalls B: cell -> p2 ----
                z1i = sm.tile([128, NB, HPC], I32, tag="z1i")
                nc.vector.tensor_copy(z1i, Z1)
                n1i = sm.tile([128, NB, HPC, 3], I32, tag="n1i")
                nc.vector.tensor_scalar(n1i[:, :, :, 0], z1i, 255, None,
                                        op0=OP.bitwise_and)
                nc.vector.tensor_scalar(n1i[:, :, :, 1], z1i, 8, 255,
                                        op0=OP.logical_shift_right,
                                        op1=OP.bitwise_and)
                nc.vector.tensor_scalar(n1i[:, :, :, 2], z1i, 16, 255,
                                        op0=OP.logical_shift_right,
                                        op1=OP.bitwise_and)
                n1 = sm.tile([128, NB, HPC, 3], F32, tag="n1")
                nc.vector.tensor_copy(n1, n1i)
                nc.vector.tensor_scalar(n1, n1, 64.0, None, op0=OP.is_lt)
                sC = sm.tile([128, NB, HPC, 2], F32, tag="sC")
                p2 = sC[:, :, :, 0]
                p2m2 = sC[:, :, :, 1]
                nc.vector.tensor_reduce(p2, n1, axis=AX.X, op=OP.add)
                nc.vector.tensor_scalar(p2, p2, 2.0, 1.0, op0=OP.mult,
                                        op1=OP.add)
                nc.vector.tensor_scalar(p2m2, p2, 2.0, None, op0=OP.subtract)

                # ---- P2b: count round 2 ----
                Z2 = sm.tile([128, NB, HPC], F32, tag="Z2")
                for i in range(NB):
                    kt = ks[i]
                    for h in range(HPC):
                        nc.vector._custom_dve(
                            KVQ_CNT3R, out=junk, in0=kt[:, h],
                            s0=p2[:, i, h: h + 1], s1=p2m2[:, i, h: h + 1],
                            imm2=256.0, accum_out=Z2[:, i, h: h + 1])

                # ---- batched smalls C: T*, T*-1, R ----
                z2i = sm.tile([128, NB, HPC], I32, tag="z2i")
                nc.vector.tensor_copy(z2i, Z2)
                yyi = sm.tile([128, NB, HPC, 3], I32, tag="yyi")
                nc.vector.tensor_scalar(yyi[:, :, :, 0], z2i, 255, None,
                                        op0=OP.bitwise_and)
                nc.vector.tensor_scalar(yyi[:, :, :, 1], z2i, 8, 255,
                                        op0=OP.logical_shift_right,
                                        op1=OP.bitwise_and)
                nc.vector.tensor_scalar(yyi[:, :, :, 2], z2i, 16, 255,
                                        op0=OP.logical_shift_right,
                                        op1=OP.bitwise_and)
                yy = sm.tile([128, NB, HPC, 3], F32, tag="yy")
                nc.vector.tensor_copy(yy, yyi)
                y0 = yy[:, :, :, 0]
                y1 = yy[:, :, :, 1]
                y2 = yy[:, :, :, 2]
                gg = sm.tile([128, NB, HPC, 2], F32, tag="gg")
                g0 = gg[:, :, :, 0]
                g1 = gg[:, :, :, 1]
                nc.vector.tensor_scalar(g0, y0, 64.0, None, op0=OP.is_ge)
                nc.vector.tensor_scalar(g1, y1, 64.0, None, op0=OP.is_ge)
                sT = sm.tile([128, NB, HPC, 3], F32, tag="sT")
                Tst = sT[:, :, :, 0]
                Tm1 = sT[:, :, :, 1]
                Rr = sT[:, :, :, 2]
                nc.vector.tensor_tensor(Tst, p2, g0, op=OP.subtract)
                nc.vector.tensor_tensor(Tst, Tst, g1, op=OP.subtract)
                nc.vector.tensor_scalar(Tst, Tst, 1.0, None, op0=OP.add)
                nc.vector.tensor_scalar(Tm1, Tst, 1.0, None, op0=OP.subtract)
                # CB = g1*y2 + (g0-g1)*y1 + (1-g0)*y0 ; R = 64 - CB
                sD = sm.tile([128, NB, HPC, 3], F32, tag="sD")
                a1 = sD[:, :, :, 0]
                a2 = sD[:, :, :, 1]
                a3 = sD[:, :, :, 2]
                nc.vector.tensor_tensor(a1, g1, y2, op=OP.mult)
                nc.vector.tensor_tensor(a2, g0, g1, op=OP.subtract)
                nc.vector.tensor_tensor(a2, a2, y1, op=OP.mult)
                nc.vector.tensor_tensor(a1, a1, a2, op=OP.add)
                nc.vector.tensor_scalar(a3, g0, -1.0, 1.0, op0=OP.mult,
                                        op1=OP.add)
                nc.vector.tensor_tensor(a3, a3, y0, op=OP.mult)
                nc.vector.tensor_tensor(a1, a1, a3, op=OP.add)
                nc.vector.tensor_scalar(Rr, a1, -1.0, 64.0, op0=OP.mult,
                                        op1=OP.add)

                # ---- P3: keep, deq, combine, store ----
                for i in range(NB):
                    ti = t0 + i
                    qt = qs[i]
                    kt = ks[i]
                    keep = tr.tile([128, HPC, D], F16, tag="keep")
                    for h in range(HPC):
                        nc.vector._custom_dve(
                            KVQ_KEEP, out=keep[:, h], in0=kt[:, h],
                            in1=Rr[:, i, h: h + 1],
                            s0=Tst[:, i, h: h + 1], s1=Tm1[:, i, h: h + 1])
                    deq = tr.tile([128, HPC, D], F32, tag="deq")
                    for h in range(HPC):
                        nc.gpsimd.tensor_scalar(
                            deq[:, h], qt[:, h], s_[:, i, h: h + 1],
                            mn[:, i, h: h + 1], op0=OP.mult, op1=OP.add)
                    kd = tr.tile([128, HPC, D], F32, tag="kd")
                    nc.gpsimd.tensor_tensor(kd, keep, deq, op=OP.mult)
                    # rowmask * kd + brep2
                    bqt = bdeq_tiles[ti // 16]
                    j = ti % 16
                    brep2 = ps.tile([128, HPC * D], F32, tag="brep2")
                    nc.tensor.matmul(
                        brep2, E_t[:, j * 128: (j + 1) * 128],
                        bqt.rearrange("p h d -> p (h d)"), start=True, stop=True)
                    outt = io.tile([128, HPC, D], F32, tag="outt")
                    nc.vector.scalar_tensor_tensor(
                        out=outt, in0=kd, scalar=rowmask,
                        in1=brep2.rearrange("p (h d) -> p h d", d=D),
                        op0=OP.mult, op1=OP.add)
                    nc.sync.dma_start(yv[ti * 128: (ti + 1) * 128], outt)
    return nc


_GRAPH_CACHE = {}


def _make_consts():
    E = np.zeros((128, 16 * 128), np.float32)
    for j in range(16):
        for m in range(128):
            E[8 * j + m // 16, j * 128 + m] = 1.0
    rowmask = np.ones((128, 1), np.float32)
    rowmask[0::16] = 0.0
    return E, rowmask


def _get_graph(S):
    if S not in _GRAPH_CACHE:
        nc = bacc.Bacc()
        build_graph(nc, S)
        nc.finalize()
        _GRAPH_CACHE[S] = nc
    return _GRAPH_CACHE[S]


LAST_RESULT = None


def kernel(feat, diff_len):
    global LAST_RESULT
    feat = np.asarray(feat)
    diff_len = int(diff_len)
    assert feat.shape == (1, H_FULL, S_FULL, D)
    E, rowmask = _make_consts()
    nc = _get_graph(S_FULL)
    in_maps = []
    for c in range(NCORES):
        shard = np.ascontiguousarray(feat[0, c * HPC: (c + 1) * HPC], np.float32)
        in_maps.append({"x": shard, "econst": E, "rowmask": rowmask})
    tkw = {}
    if os.environ.get("KVQ_TRACE"):
        tkw = dict(trace=True, tmpdir=os.environ.get("KVQ_TRACE_DIR") or None)
    res = run_bass_kernel_spmd(nc, in_maps, core_ids=list(range(NCORES)), **tkw)
    LAST_RESULT = res
    outs = [np.asarray(r["y"]).reshape(HPC, S_FULL, D) for r in res.results]
    full = np.concatenate(outs, axis=0)[None]   # [1, 32, 8192, 128]
    if diff_len < S_FULL:
        full = full.copy()
        full[:, :, diff_len:] = feat[:, :, diff_len:]
    return full.astype(feat.dtype, copy=False)


# revision 11
# speedup vs baseline: 27259.2802x; 1.3726x over previous
"""KVQuantizer Trainium2 kernel (v2 — custom-DVE selection).

Full input feat [1, 32, 8192, 128] fp32 sharded head-wise across 8 cores
(4 heads/core). Per (token, head): 128-dim group quantization:
  - chunk (16 tokens) base row quantized at 8 bits (asymmetric per-group)
  - diffs vs dequantized base quantized at 4 bits + top-k pruning
    (zero the 64 smallest-|deq| per group, jax top_k tie semantics)
  - out = base_deq + pruned diff_deq

Selection strategy: |deq| = s*|q - c| with c = -b/s, so ranking reduces to
an integer "level index" kappa = 2*m + side-bit per element (m = integer
distance from c), computed per head by a fused custom-DVE op. The 64-of-128
threshold T* is found from packed 3x8-bit CDF counts (two rounds), and the
exact tie order (lowest index first) comes from an in-op prefix scan.
Verified bit-exact against jax top_k on the seed-0 dataset.
"""
import os
import sys
import numpy as np

sys.path.insert(0, "/opt/trn_rl_repo")

import concourse.bass as bass
import concourse.bacc as bacc
import concourse.mybir as mybir
from concourse.tile import TileContext
from concourse.bass_utils import run_bass_kernel_spmd
from concourse import dve_ops
from concourse.dve_ops import DveOp
from concourse.dve_uop import DveOpSpec
from concourse.dve_spec import (
    Spec, Src0, Src1, C0, C1, C2, C3, Zero, One, relu, sq, minn, select,
    lower, AluOp, scan, _spill_c3_to_src1, _has_src1,
)

F32 = mybir.dt.float32
F16 = mybir.dt.float16
I32 = mybir.dt.int32
AF = mybir.ActivationFunctionType
OP = mybir.AluOpType
AX = mybir.AxisListType

NCORES = 8
H_FULL = 32
HPC = H_FULL // NCORES   # heads per core = 4
S_FULL = 8192
D = 128
CH = 16                  # chunk size
EPS = 1e-5
NB = 16                  # tiles per super-batch
MAGIC = float(np.float32(12582912.0))        # 1.5 * 2^23
H_CEIL = float(np.float32(0.5 - 2**-16))     # RNE(x+this) = ceil(x)


# ---------------------------------------------------------------------------
# custom DVE ops
# ---------------------------------------------------------------------------

def _mk_op(name, spec, subdim=False):
    if name in dve_ops._SUB_OPCODE_FOR_NAME:
        for op in dve_ops.OPS:
            if op.name == name:
                return op
    shas = {}
    for ver in ("v3", "v4"):
        s = DveOpSpec(name=name, opcode=1, uops=lower(spec, ver=ver),
                      rd1_en=_has_src1(spec))
        shas[ver] = s.sha(ver)
    op = DveOp(name, spec, subdim, shas)
    dve_ops.OPS.append(op)
    dve_ops._SUB_OPCODE_FOR_NAME[name] = (
        dve_ops._CUSTOM_DVE_ROW_BASE + len(dve_ops.OPS) - 1)
    dve_ops.CUSTOM_DVE_SPECS[name] = op.spec
    assert dve_ops._SUB_OPCODE_FOR_NAME[name] < 0x20
    return op


def _ref_q(in0, in1, s0, s1, imm2):
    v = (in0.astype(np.float32) - s0) * s1
    r = ((v + imm2) - imm2).astype(np.float32)
    return np.minimum(np.maximum(r, 0.0), in1).astype(np.float32)


def _ref_kappa(in0, in1, s0, s1, imm2):
    t = in0.astype(np.float32) * 2.0
    u1 = t - s0
    u2 = s1 - t
    return np.where(u1 >= 0, u1, u2).astype(np.float32)


def _ref_cnt3i(in0, in1, s0, s1, imm2):
    x = in0.astype(np.float32)
    e = ((x <= s0) + ((x <= s1) + (x <= imm2) * in1) * in1).astype(np.float32)
    return e, e.reshape(e.shape[0], -1).sum(-1, keepdims=True)


def _ref_cnt3r(in0, in1, s0, s1, imm2):
    x = in0.astype(np.float32)
    e = ((x <= s0) + ((x <= s0 - 1) + (x <= s1) * imm2) * imm2).astype(np.float32)
    return e, e.reshape(e.shape[0], -1).sum(-1, keepdims=True)


def _ref_keep(in0, in1, s0, s1, imm2):
    x = in0.astype(np.float32)
    a_le = (x <= s0).astype(np.float32)
    b_lt = (x <= s1).astype(np.float32)
    tie = a_le - b_lt
    P = np.cumsum(tie, axis=-1)
    return (1.0 - (b_lt + tie * (P <= in1))).astype(np.float32)


# q = clip(rne((diff - b) * rs), 0, 15); C0=b, C1=rs, C2=MAGIC, Src1=15-const
KVQ_Q = _mk_op("KVQ_Q", Spec(
    body=_spill_c3_to_src1(minn(relu(((Src0 - C0) * C1 + C2) - C2), C3)),
    reference=_ref_q))

# kappa = select(2q - A >= 0, 2q - A, B - 2q); C0=A, C1=B
_t2 = Src0 + Src0
_u1 = _t2 - C0
_u2 = C1 - _t2
KVQ_KAPPA = _mk_op("KVQ_KAPPA", Spec(
    body=select(_u1 >= Zero, _u1, _u2),
    reference=_ref_kappa))

# packed counts at imm thresholds C0, C1, C2 (fields 1, 256, 65536); Src1=256
KVQ_CNT3I = _mk_op("KVQ_CNT3I", Spec(
    body=_spill_c3_to_src1(
        (Src0 <= C0) + ((Src0 <= C1) + (Src0 <= C2) * C3) * C3),
    accum=AluOp.ADD,
    reference=_ref_cnt3i))

# packed counts at AP thresholds C0, C0-1, C1; multiplier C2 (=256 imm)
KVQ_CNT3R = _mk_op("KVQ_CNT3R", Spec(
    body=(Src0 <= C0) + ((Src0 <= C0 - One) + (Src0 <= C1) * C2) * C2,
    accum=AluOp.ADD,
    reference=_ref_cnt3r))

# keep = 1 - (b_lt + tie * (prefix_count(tie) <= R)); C0=T*, C1=T*-1, Src1=R
_a_le = Src0 <= C0
_b_lt = Src0 <= C1
_tie = _a_le - _b_lt
_P = scan(AluOp.ADD, _tie)
KVQ_KEEP = _mk_op("KVQ_KEEP", Spec(
    body=_spill_c3_to_src1(One - (_b_lt + _tie * (_P <= C3))),
    reference=_ref_keep))


def _ref_qb(in0, in1, s0, s1, imm2):
    r = ((in0.astype(np.float32) + imm2) - imm2).astype(np.float32)
    return np.minimum(np.maximum(r, 0.0), 15.0).astype(np.float32)


def _ref_kb(in0, in1, s0, s1, imm2):
    u1 = (in0.astype(np.float32) * 2.0 - in1).astype(np.float32)
    return np.where(u1 >= 0, u1, -u1 - 1.0).astype(np.float32)


_two = One + One
_fifteen = sq(sq(_two)) - One

# q = clip(rne(Src0), 0, 15); C2=MAGIC  (big op; v precomputed on ScalarE)
KVQ_QB = _mk_op("KVQ_QB", Spec(
    body=minn(relu((Src0 + C2) - C2), _fifteen),
    reference=_ref_qb))

# kappa = select(2q - A >= 0, 2q - A, -(2q - A) - 1); Src1 = A broadcast
_u1b = (Src0 + Src0) - Src1
KVQ_KB = _mk_op("KVQ_KB", Spec(
    body=select(_u1b >= Zero, _u1b, Zero - _u1b - One),
    reference=_ref_kb))


# ---------------------------------------------------------------------------
# graph
# ---------------------------------------------------------------------------

def build_graph(nc, S=S_FULL):
    n_chunks = S // CH
    n_btiles = (n_chunks + 127) // 128
    n_tiles = S // 128
    n_sb = (n_tiles + NB - 1) // NB
    assert n_tiles % NB == 0

    x = nc.dram_tensor("x", [HPC, S, D], F32, kind="ExternalInput")
    econst = nc.dram_tensor("econst", [128, 16 * 128], F32, kind="ExternalInput")
    rowmask_d = nc.dram_tensor("rowmask", [128, 1], F32, kind="ExternalInput")
    y = nc.dram_tensor("y", [HPC, S, D], F32, kind="ExternalOutput")

    xv = x[:, :, :].rearrange("h s d -> s h d")
    yv = y[:, :, :].rearrange("h s d -> s h d")
    xb = x[:, :, :].rearrange("h (c j) d -> c j h d", j=CH)

    with TileContext(nc) as tc:
        with (
            tc.tile_pool(name="consts", bufs=1) as cpool,
            tc.tile_pool(name="bdq", bufs=1) as bdpool,
            tc.tile_pool(name="io", bufs=3) as io,
            tc.tile_pool(name="wk", bufs=2) as wk,
            tc.tile_pool(name="diffp", bufs=1) as diffp,
            tc.tile_pool(name="qp", bufs=2) as qp,
            tc.tile_pool(name="kp", bufs=2) as kp,
            tc.tile_pool(name="sm", bufs=2) as sm,
            tc.tile_pool(name="tr", bufs=3) as tr,
            tc.tile_pool(name="ps", bufs=3, space="PSUM") as ps,
        ):
            E_t = cpool.tile([128, 16 * 128], F32, tag="E")
            nc.sync.dma_start(E_t, econst[:, :])
            rowmask = cpool.tile([128, 1], F32, tag="rowmask")
            nc.sync.dma_start(rowmask, rowmask_d[:, :])
            c15 = cpool.tile([128, 1], F32, tag="c15")
            nc.vector.memset(c15, 15.0)
            c256 = cpool.tile([128, 1], F32, tag="c256")
            nc.vector.memset(c256, 256.0)
            q255_t = cpool.tile([128, 1], F32, tag="q255")
            nc.vector.memset(q255_t, 255.0)

            # ---------- pass 0: chunk-base rows, 8-bit quant (as v1) ----------
            bdeq_tiles = []
            for bt in range(n_btiles):
                nb = min(128, n_chunks - bt * 128)
                xt = io.tile([128, HPC, D], F32, tag="xin")
                nc.sync.dma_start(xt[:nb], xb[bt * 128: bt * 128 + nb, 0])
                red = wk.tile([128, HPC, 2], F32, tag="bred")
                rmax = red[:, :, 0]
                rmin = red[:, :, 1]
                nc.vector.tensor_reduce(rmax, xt[:nb], axis=AX.X, op=OP.max)
                nc.vector.tensor_reduce(rmin, xt[:nb], axis=AX.X, op=OP.min)
                sc = wk.tile([128, HPC, 2], F32, tag="bscales")
                s_b = sc[:, :, 0]
                rs_b = sc[:, :, 1]
                nc.vector.tensor_tensor(s_b, rmax, rmin, op=OP.subtract)
                nc.vector.tensor_scalar(s_b, s_b, 1.0 / 255.0, EPS,
                                        op0=OP.mult, op1=OP.max)
                nc.vector.reciprocal(rs_b, s_b)
                qt = wk.tile([128, HPC, D], F32, tag="bq")
                for h in range(HPC):
                    nc.vector._custom_dve(
                        KVQ_Q, out=qt[:nb, h], in0=xt[:nb, h],
                        in1=q255_t[:nb], s0=rmin[:, h: h + 1],
                        s1=rs_b[:, h: h + 1], imm2=MAGIC)
                bq = bdpool.tile([128, HPC, D], F32, tag=f"bdeq{bt}")
                for h in range(HPC):
                    nc.vector.tensor_scalar(
                        bq[:nb, h], qt[:nb, h], s_b[:, h: h + 1],
                        rmin[:, h: h + 1], op0=OP.mult, op1=OP.add)
                bdeq_tiles.append(bq)

            # ---------- main: super-batches of NB tiles ----------
            for sb in range(n_sb):
                t0 = sb * NB
                # stats tiles for this batch (contiguous [128, HPC] writes)
                stq = sm.tile([128, 2, NB, HPC], F32, tag="stq")   # mx, mn
                diffs = []
                for i in range(NB):
                    ti = t0 + i
                    xt = io.tile([128, HPC, D], F32, tag="xin")
                    nc.sync.dma_start(xt, xv[ti * 128: (ti + 1) * 128])
                    bqt = bdeq_tiles[ti // 16]
                    j = ti % 16
                    brep = ps.tile([128, HPC * D], F32, tag="brep")
                    nc.tensor.matmul(
                        brep, E_t[:, j * 128: (j + 1) * 128],
                        bqt.rearrange("p h d -> p (h d)"), start=True, stop=True)
                    dt = diffp.tile([128, HPC, D], F32, tag=f"diff{i}")
                    nc.vector.tensor_tensor(
                        dt, xt, brep.rearrange("p (h d) -> p h d", d=D),
                        op=OP.subtract)
                    nc.vector.tensor_reduce(stq[:, 0, i, :], dt, axis=AX.X,
                                            op=OP.max)
                    nc.vector.tensor_reduce(stq[:, 1, i, :], dt, axis=AX.X,
                                            op=OP.min)
                    diffs.append(dt)

                # ---- batched smalls A: quant chain ----
                mx = stq[:, 0]
                mn = stq[:, 1]
                sA = sm.tile([128, NB, HPC, 4], F32, tag="sA")  # s, rs, c, eqf
                s_ = sA[:, :, :, 0]
                rs = sA[:, :, :, 1]
                c = sA[:, :, :, 2]
                eqf = sA[:, :, :, 3]
                nc.vector.tensor_tensor(s_, mx, mn, op=OP.subtract)
                nc.vector.tensor_scalar(s_, s_, 1.0 / 15.0, EPS,
                                        op0=OP.mult, op1=OP.max)
                nc.vector.reciprocal(rs, s_)
                nc.vector.scalar_tensor_tensor(
                    out=c, in0=mn, scalar=-1.0, in1=rs, op0=OP.mult, op1=OP.mult)
                sB = sm.tile([128, NB, HPC, 4], F32, tag="sB")
                ff = sB[:, :, :, 0]
                cc = sB[:, :, :, 1]
                f1 = sB[:, :, :, 2]
                AA = sB[:, :, :, 3]
                nc.vector.tensor_scalar(ff, c, -H_CEIL, MAGIC, op0=OP.add,
                                        op1=OP.add)
                nc.vector.tensor_scalar(ff, ff, MAGIC, None, op0=OP.subtract)
                nc.vector.tensor_scalar(cc, c, H_CEIL, MAGIC, op0=OP.add,
                                        op1=OP.add)
                nc.vector.tensor_scalar(cc, cc, MAGIC, None, op0=OP.subtract)
                nc.vector.tensor_tensor(f1, cc, c, op=OP.subtract)
                nc.vector.tensor_tensor(ff, c, ff, op=OP.subtract)  # ff <- f2
                nc.vector.tensor_tensor(eqf, f1, ff, op=OP.is_equal)
                nc.vector.tensor_tensor(f1, f1, ff, op=OP.is_gt)    # f1 <- gt
                # bA = gt + 0.5*eqf (integer-c groups are zero rows; ignored)
                nc.vector.scalar_tensor_tensor(
                    out=f1, in0=eqf, scalar=0.5, in1=f1, op0=OP.mult, op1=OP.add)
                nc.vector.scalar_tensor_tensor(
                    out=AA, in0=cc, scalar=2.0, in1=f1, op0=OP.mult,
                    op1=OP.subtract)

                # ---- P2: v (ScalarE), q, kappa, counts ----
                Z1 = sm.tile([128, NB, HPC], F32, tag="Z1")
                Z2 = sm.tile([128, NB, HPC], F32, tag="Z2")
                qs = []
                ks = []
                for i in range(NB):
                    dt = diffs[i]
                    vt = tr.tile([128, HPC, D], F32, tag="vt")
                    for h in range(HPC):
                        nc.scalar.activation(
                            vt[:, h], dt[:, h], AF.Identity,
                            bias=c[:, i, h: h + 1], scale=rs[:, i, h: h + 1])
                    qt = qp.tile([128, HPC, D], F16, tag=f"q{i}")
                    nc.vector._custom_dve(KVQ_QB, out=qt, in0=vt, imm2=MAGIC)
                    kt = kp.tile([128, HPC, D], F16, tag=f"k{i}")
                    nc.vector._custom_dve(
                        KVQ_KB, out=kt, in0=qt,
                        in1=AA[:, i, :].to_broadcast([128, HPC, D]))
                    e1 = tr.tile([128, HPC, D], F32, tag="e1")
                    nc.vector._custom_dve(
                        KVQ_CNT3I, out=e1, in0=kt, in1=c256,
                        s0=1.0, s1=3.0, imm2=5.0)
                    nc.vector.tensor_reduce(Z1[:, i, :], e1, axis=AX.X,
                                            op=OP.add)
                    e2 = tr.tile([128, HPC, D], F32, tag="e2")
                    nc.vector._custom_dve(
                        KVQ_CNT3I, out=e2, in0=kt, in1=c256,
                        s0=2.0, s1=4.0, imm2=6.0)
                    nc.vector.tensor_reduce(Z2[:, i, :], e2, axis=AX.X,
                                            op=OP.add)
                    qs.append(qt)
                    ks.append(kt)

                # ---- batched smalls B: T*, T*-1, R from 6-point CDF ----
                z1i = sm.tile([128, NB, HPC], I32, tag="z1i")
                z2i = sm.tile([128, NB, HPC], I32, tag="z2i")
                nc.vector.tensor_copy(z1i, Z1)
                nc.vector.tensor_copy(z2i, Z2)
                n6i = sm.tile([128, NB, HPC, 2, 3], I32, tag="n6i")
                nc.vector.tensor_scalar(n6i[:, :, :, 0, 0], z1i, 255, None,
                                        op0=OP.bitwise_and)
                nc.vector.tensor_scalar(n6i[:, :, :, 0, 1], z1i, 8, 255,
                                        op0=OP.logical_shift_right,
                                        op1=OP.bitwise_and)
                nc.vector.tensor_scalar(n6i[:, :, :, 0, 2], z1i, 16, 255,
                                        op0=OP.logical_shift_right,
                                        op1=OP.bitwise_and)
                nc.vector.tensor_scalar(n6i[:, :, :, 1, 0], z2i, 255, None,
                                        op0=OP.bitwise_and)
                nc.vector.tensor_scalar(n6i[:, :, :, 1, 1], z2i, 8, 255,
                                        op0=OP.logical_shift_right,
                                        op1=OP.bitwise_and)
                nc.vector.tensor_scalar(n6i[:, :, :, 1, 2], z2i, 16, 255,
                                        op0=OP.logical_shift_right,
                                        op1=OP.bitwise_and)
                n6 = sm.tile([128, NB, HPC, 2, 3], F32, tag="n6")
                nc.vector.tensor_copy(n6, n6i)
                ge = sm.tile([128, NB, HPC, 2, 3], F32, tag="ge")
                nc.vector.tensor_scalar(ge, n6, 64.0, None, op0=OP.is_ge)
                sT = sm.tile([128, NB, HPC, 3], F32, tag="sT")
                Tst = sT[:, :, :, 0]
                Tm1 = sT[:, :, :, 1]
                Rr = sT[:, :, :, 2]
                nc.vector.tensor_reduce(Tst, ge, axis=AX.XY, op=OP.add)
                nc.vector.tensor_scalar(Tst, Tst, -1.0, 7.0, op0=OP.mult,
                                        op1=OP.add)
                nc.vector.tensor_scalar(Tm1, Tst, 1.0, None, op0=OP.subtract)
                # CB = max(n6 * (1 - ge)); R = 64 - CB
                nc.vector.tensor_scalar(ge, ge, -1.0, 1.0, op0=OP.mult,
                                        op1=OP.add)
                nc.vector.tensor_tensor(ge, ge, n6, op=OP.mult)
                nc.vector.tensor_reduce(Rr, ge, axis=AX.XY, op=OP.max)
                nc.vector.tensor_scalar(Rr, Rr, -1.0, 64.0, op0=OP.mult,
                                        op1=OP.add)
                # rowmask folded into deq: s_rm = s * rowmask, b_rm = b * rowmask
                srm = sm.tile([128, NB, HPC, 2], F32, tag="srm")
                s_rm = srm[:, :, :, 0]
                b_rm = srm[:, :, :, 1]
                nc.vector.tensor_scalar(s_rm, s_, rowmask, None, op0=OP.mult)
                nc.vector.tensor_scalar(b_rm, mn, rowmask, None, op0=OP.mult)

                # ---- P3: keep, deq, combine, store ----
                for i in range(NB):
                    ti = t0 + i
                    qt = qs[i]
                    kt = ks[i]
                    keep = tr.tile([128, HPC, D], F16, tag="keep")
                    for h in range(HPC):
                        nc.vector._custom_dve(
                            KVQ_KEEP, out=keep[:, h], in0=kt[:, h],
                            in1=Rr[:, i, h: h + 1],
                            s0=Tst[:, i, h: h + 1], s1=Tm1[:, i, h: h + 1])
                    deq = tr.tile([128, HPC, D], F32, tag="deq")
                    for h in range(HPC):
                        nc.scalar.activation(
                            deq[:, h], qt[:, h], AF.Identity,
                            bias=b_rm[:, i, h: h + 1],
                            scale=s_rm[:, i, h: h + 1])
                    kd = tr.tile([128, HPC, D], F32, tag="kd")
                    nc.gpsimd.tensor_tensor(kd, keep, deq, op=OP.mult)
                    bqt = bdeq_tiles[ti // 16]
                    j = ti % 16
                    brep2 = ps.tile([128, HPC * D], F32, tag="brep2")
                    nc.tensor.matmul(
                        brep2, E_t[:, j * 128: (j + 1) * 128],
                        bqt.rearrange("p h d -> p (h d)"), start=True, stop=True)
                    outt = io.tile([128, HPC, D], F32, tag="outt")
                    nc.vector.tensor_tensor(
                        outt, kd, brep2.rearrange("p (h d) -> p h d", d=D),
                        op=OP.add)
                    nc.sync.dma_start(yv[ti * 128: (ti + 1) * 128], outt)
    return nc


_GRAPH_CACHE = {}


def _make_consts():
    E = np.zeros((128, 16 * 128), np.float32)
    for j in range(16):
        for m in range(128):
            E[8 * j + m // 16, j * 128 + m] = 1.0
    rowmask = np.ones((128, 1), np.float32)
    rowmask[0::16] = 0.0
    return E, rowmask


def _get_graph(S):
    if S not in _GRAPH_CACHE:
        nc = bacc.Bacc()
        build_graph(nc, S)
        nc.finalize()
        _GRAPH_CACHE[S] = nc
    return _GRAPH_CACHE[S]


LAST_RESULT = None


def kernel(feat, diff_len):
    global LAST_RESULT
    feat = np.asarray(feat)
    diff_len = int(diff_len)
    assert feat.shape == (1, H_FULL, S_FULL, D)
    E, rowmask = _make_consts()
    nc = _get_graph(S_FULL)
    in_maps = []
    for c in range(NCORES):
        shard = np.ascontiguousarray(feat[0, c * HPC: (c + 1) * HPC], np.float32)
        in_maps.append({"x": shard, "econst": E, "rowmask": rowmask})
    tkw = {}
    if os.environ.get("KVQ_TRACE"):
        tkw = dict(trace=True, tmpdir=os.environ.get("KVQ_TRACE_DIR") or None)
    res = run_bass_kernel_spmd(nc, in_maps, core_ids=list(range(NCORES)), **tkw)
    LAST_RESULT = res
    outs = [np.asarray(r["y"]).reshape(HPC, S_FULL, D) for r in res.results]
    full = np.concatenate(outs, axis=0)[None]   # [1, 32, 8192, 128]
    if diff_len < S_FULL:
        full = full.copy()
        full[:, :, diff_len:] = feat[:, :, diff_len:]
    return full.astype(feat.dtype, copy=False)
